# revision 11
# baseline (speedup 1.0000x reference)
"""DMPNN (NNConv edge-network message passing) Trainium2 kernel, 8-core SPMD.

Algorithm: instead of materializing per-edge [H,H] weights (the reference's
W_e = relu(ea@e1)@e2, then msg_e = h_src W_e, scatter-mean), contract edges
into dst nodes FIRST via per-node outer products:

  C[n, (k,h)] = sum_{e->n} ev[e,k] * h_src[e,h] / deg_n      (tiny PE matmuls)
  aggT[o, n]  = sum_{k,h} e2w[k,h,o] * C[n,(k,h)]            (dense PE matmul)
              + hbar_n @ e2b + h @ root_w                     (bias + root)

This cuts the big contraction from E to N columns and avoids any E x 16384
intermediate. Per-node matmuls use 32-row PE array tiling: each dst node
group of 4 ("quadrant", 32 slots) does ONE [32]x[32,512] matmul with
color-masked ev (mask zeroes other nodes' rows; applied free during the
relu eviction).

Sharding: dst-node range per core (512 nodes). Per layer: AllReduce of BN
stats + AllGather of updated node features.
"""

import numpy as np
import ml_dtypes

import concourse.bass as bass
import concourse.tile as tile
import concourse.mybir as mybir
from concourse import bacc
from concourse.bass import IndirectOffsetOnAxis
from concourse.bass_utils import run_bass_kernel_spmd

BF16 = ml_dtypes.bfloat16

N, E, F_NODE, F_EDGE, H, L, G = 4096, 12288, 64, 16, 128, 4, 256
NC = 8
NS = N // NC            # nodes per core (512)
P = 128
T = 32                  # slot tiles per core (32 x 128 slots)
NQ = 4 * T              # quadrants (one per 4 nodes)
NCHUNK = 4              # agg chunks of 128 nodes
BN_EPS = 1e-5
AXF = mybir.ActivationFunctionType
ALU = mybir.AluOpType


# ----------------------------------------------------------------------------
# Host preprocessing
# ----------------------------------------------------------------------------

def _preprocess(edge_index, edge_attr):
    src = np.asarray(edge_index[0], dtype=np.int64)
    dst = np.asarray(edge_index[1], dtype=np.int64)
    ea = np.asarray(edge_attr, dtype=np.float32)
    deg = np.bincount(dst, minlength=N).astype(np.float32)
    inv_deg = np.where(deg > 0, 1.0 / np.maximum(deg, 1.0), 0.0).astype(np.float32)

    # edges grouped by dst
    order = np.argsort(dst, kind="stable")
    starts = np.searchsorted(dst[order], np.arange(N))
    ends = np.searchsorted(dst[order], np.arange(N), side="right")

    cores = []
    for c in range(NC):
        eaT = np.zeros((17, T * P), np.float32)
        srcg = np.zeros((P, T), np.int32)
        mscale = np.zeros((P, T * 4), np.float32)
        smat = np.zeros((P, T * 16), np.float32)
        for t in range(T):
            for q in range(4):
                p0 = 32 * q
                fill = 0
                for cidx in range(4):
                    n_local = 16 * t + 4 * q + cidx
                    n_glob = c * NS + n_local
                    es = order[starts[n_glob]:ends[n_glob]]
                    k = len(es)
                    assert fill + k <= 32, (c, t, q, fill, k)
                    sl = slice(p0 + fill, p0 + fill + k)
                    eaT[:F_EDGE, t * P + p0 + fill: t * P + p0 + fill + k] = ea[es].T
                    eaT[F_EDGE, t * P + p0 + fill: t * P + p0 + fill + k] = 1.0
                    srcg[sl, t] = src[es]
                    mscale[sl, t * 4 + cidx] = inv_deg[n_glob]
                    smat[sl, t * 16 + 4 * q + cidx] = inv_deg[n_glob]
                    fill += k
        cores.append(dict(eaT=eaT, srcg=srcg, mscale=mscale, smat=smat))
    return cores


# ----------------------------------------------------------------------------
# Device program
# ----------------------------------------------------------------------------

def _build():
    f32 = mybir.dt.float32
    bf16 = mybir.dt.bfloat16
    i32 = mybir.dt.int32
    nc = bacc.Bacc("TRN2", target_bir_lowering=False, debug=False, num_devices=NC)

    def din(name, shape, dt=bf16):
        return nc.dram_tensor(name, shape, dt, kind="ExternalInput")

    ea_d = din("ea", [17, T * P])
    srcg_d = din("srcg", [P, T], i32)
    mscale_d = din("mscale", [P, T * 4], f32)
    smat_d = din("smat", [P, T * 16])
    e1w_d = din("e1w", [L, 17, H])
    w2s_d = din("w2s", [L, H, H * H])      # e2w_stack[l][h, k*128+o]
    b2s_d = din("b2s", [L, H, H])          # e2_b as [h, o]
    rw_d = din("rw", [L, H, H])            # root_w as [h, o]
    bng_d = din("bng", [L, H, 1], f32)
    bnb_d = din("bnb", [L, H, 1], f32)
    xs_d = din("xs", [65, NS])
    nw_d = din("nw", [65, H])
    pmat_d = din("pmat", [P, 4 * G])
    hw1_d = din("hw1", [H, H])
    hb1_d = din("hb1", [H, 1], f32)
    hw2_d = din("hw2", [H, 1])
    hb2_d = din("hb2", [1, 1], f32)
    idf_d = din("idf", [P, P], f32)
    y_d = nc.dram_tensor("y", [1, G], f32, kind="ExternalOutput")

    groups = [list(range(NC))]

    with tile.TileContext(nc) as tc:
        with tc.tile_pool(name="const", bufs=1) as const, \
             tc.tile_pool(name="persist", bufs=1) as persist, \
             tc.tile_pool(name="w2pool", bufs=1) as w2pool, \
             tc.tile_pool(name="cpool", bufs=1) as cpool, \
             tc.tile_pool(name="evpool", bufs=2) as evpool, \
             tc.tile_pool(name="hspool", bufs=1) as hspool, \
             tc.tile_pool(name="spool", bufs=2) as spool, \
             tc.tile_pool(name="stat", bufs=2) as statp, \
             tc.tile_pool(name="psc", bufs=2, space="PSUM") as ps_c, \
             tc.tile_pool(name="psagg", bufs=1, space="PSUM") as ps_agg, \
             tc.tile_pool(name="pshb", bufs=1, space="PSUM") as ps_hb, \
             tc.tile_pool(name="psev", bufs=2, space="PSUM") as ps_ev, \
             tc.tile_pool(name="dramp", bufs=2, space="DRAM") as dramp:

            # ---- persistent constants ----
            ea_sb = const.tile([17, T * P], bf16)
            nc.sync.dma_start(ea_sb[:], ea_d[:])
            srcg_sb = const.tile([P, T], i32)
            nc.sync.dma_start(srcg_sb[:], srcg_d[:])
            mscale_sb = const.tile([P, T * 4], f32)
            nc.sync.dma_start(mscale_sb[:], mscale_d[:])
            smat_sb = const.tile([P, T * 16], bf16)
            nc.sync.dma_start(smat_sb[:], smat_d[:])
            xs_sb = const.tile([65, NS], bf16)
            nc.sync.dma_start(xs_sb[:], xs_d[:])
            nw_sb = const.tile([65, H], bf16)
            nc.sync.dma_start(nw_sb[:], nw_d[:])
            idf_sb = const.tile([P, P], f32)
            nc.sync.dma_start(idf_sb[:], idf_d[:])
            e1w_sb, b2_sb, rw_sb, bng_sb, bnb_sb = [], [], [], [], []
            for l in range(L):
                t_ = const.tile([17, H], bf16, name=f"e1w_{l}")
                nc.sync.dma_start(t_[:], e1w_d[l])
                e1w_sb.append(t_)
                t_ = const.tile([H, H], bf16, name=f"b2_{l}")
                nc.sync.dma_start(t_[:], b2s_d[l])
                b2_sb.append(t_)
                t_ = const.tile([H, H], bf16, name=f"rw_{l}")
                nc.sync.dma_start(t_[:], rw_d[l])
                rw_sb.append(t_)
                t_ = const.tile([H, 1], f32, name=f"bng_{l}")
                nc.sync.dma_start(t_[:], bng_d[l])
                bng_sb.append(t_)
                t_ = const.tile([H, 1], f32, name=f"bnb_{l}")
                nc.sync.dma_start(t_[:], bnb_d[l])
                bnb_sb.append(t_)
            eps_sb = const.tile([H, 1], f32)
            nc.vector.memset(eps_sb[:], BN_EPS)

            hT = persist.tile([H, NS], f32)      # own node features, [h, n]
            hrow_last = [persist.tile([P, H], bf16, name=f"hrl_{j}")
                         for j in range(4)]

            # ---- node encoder (own slice only) + AllGather ----
            hsl0 = dramp.tile([NS, H], bf16, name="hsl0", bufs=1)
            with tc.tile_pool(name="encp", bufs=2) as encp:
                for j in range(NS // P):
                    ps = ps_ev.tile([P, H], f32, name="enc_ps", tag="e")
                    nc.tensor.matmul(ps[:], xs_sb[:, j * P:(j + 1) * P], nw_sb[:],
                                     start=True, stop=True)
                    tmp = encp.tile([P, H], f32, name="enc_tmp")
                    nc.vector.tensor_copy(tmp[:], ps[:])
                    hrow = encp.tile([P, H], bf16, name="enc_row")
                    nc.scalar.copy(hrow[:], ps[:])
                    nc.sync.dma_start(hsl0[j * P:(j + 1) * P, :], hrow[:])
                    ps2 = ps_ev.tile([P, P], f32, name="enc_ps2", tag="e")
                    nc.tensor.transpose(ps2[:], tmp[:], idf_sb[:])
                    nc.scalar.copy(hT[:, j * P:(j + 1) * P], ps2[:])
            hfull0 = dramp.tile([N, H], bf16, name="hfull0", bufs=1)
            nc.gpsimd.collective_compute(
                "AllGather", ALU.bypass, replica_groups=groups,
                ins=[hsl0.opt()], outs=[hfull0.opt()])
            h_rows = hfull0

            # ev generation + color-masked eviction (scaled by 1/deg).
            # Depends only on edge_attr, so layer l+1's ev runs during
            # layer l's BN AllReduce wait.
            def gen_ev(l):
                ev_all = evpool.tile([P, T * 512], bf16, name="ev_all")
                for t in range(T):
                    evps = ps_ev.tile([P, H], f32, name="evps", tag="e")
                    nc.tensor.matmul(evps[:], ea_sb[:17, t * P:(t + 1) * P],
                                     e1w_sb[l][:], start=True, stop=True)
                    for cidx in range(4):
                        dst_ap = ev_all[:, t * 512 + cidx * P:
                                        t * 512 + (cidx + 1) * P]
                        sc = mscale_sb[:, t * 4 + cidx: t * 4 + cidx + 1]
                        if cidx % 2 == 0:
                            nc.vector.tensor_scalar(
                                out=dst_ap, in0=evps[:], scalar1=sc,
                                scalar2=0.0, op0=ALU.mult, op1=ALU.max)
                        else:
                            nc.scalar.activation(dst_ap, evps[:], AXF.Relu,
                                                 scale=sc)
                return ev_all

            ev_next = gen_ev(0)

            # ---- layers ----
            for l in range(L):
                ev_all = ev_next
                w2_sb = w2pool.tile([H, H * H], bf16, name="w2")
                nc.sync.dma_start(w2_sb[:], w2s_d[l])

                hTb = spool.tile([H, NS], bf16, name="hTb")
                nc.vector.tensor_copy(hTb[:], hT[:])

                # gather h_src rows for all slots
                hs_all = hspool.tile([P, T * H], bf16, name="hs_all")
                for t in range(T):
                    nc.gpsimd.indirect_dma_start(
                        out=hs_all[:, t * H:(t + 1) * H], out_offset=None,
                        in_=h_rows[:],
                        in_offset=IndirectOffsetOnAxis(
                            ap=srcg_sb[:, t:t + 1], axis=0))

                # hbar[h, n] = scatter-mean of h_src (for the e2_b term)
                hbps = ps_hb.tile([H, NS], f32, name="hbps")
                for t in range(T):
                    nc.tensor.matmul(hbps[:, t * 16:(t + 1) * 16],
                                     hs_all[:, t * H:(t + 1) * H],
                                     smat_sb[:, t * 16:(t + 1) * 16],
                                     start=True, stop=True,
                                     skip_group_check=True)
                hbarT = spool.tile([H, NS], bf16, name="hbarT")
                nc.vector.tensor_copy(hbarT[:], hbps[:])

                # Per 256-node pair: C-build (one [32]x[32,512] matmul per
                # quadrant, two quadrants per 2-bank psum tile), then agg
                # with 256 moving columns per w2 weight-load.
                aggps = ps_agg.tile([H, NS], f32, name="aggps", tag="a")
                for pr in range(NCHUNK // 2):
                    cc = cpool.tile([H, 256 * H], bf16, name="cc")
                    for tt in range(16):
                        t = pr * 16 + tt
                        for qp in range(2):
                            cps = ps_c.tile([H, 1024], f32, name="cps")
                            for qi in range(2):
                                q = qp * 2 + qi
                                nc.tensor.matmul(
                                    cps[:, qi * 512:(qi + 1) * 512],
                                    hs_all[32 * q:32 * (q + 1),
                                           t * H:(t + 1) * H],
                                    ev_all[32 * q:32 * (q + 1),
                                           t * 512:(t + 1) * 512],
                                    start=True, stop=True,
                                    tile_position=(32 * q, 0),
                                    skip_group_check=True)
                            off = (tt * 16 + 8 * qp) * P
                            if (tt + qp) % 2 == 0:
                                nc.vector.tensor_copy(
                                    cc[:, off:off + 1024], cps[:])
                            else:
                                nc.scalar.copy(cc[:, off:off + 1024],
                                               cps[:])
                    cv = cc[:].rearrange("p (n k) -> p n k", n=256)
                    dst_ap = aggps[:, pr * 256:(pr + 1) * 256]
                    for k in range(H):
                        nc.tensor.matmul(dst_ap, w2_sb[:, k * P:(k + 1) * P],
                                         cv[:, :, k], start=(k == 0),
                                         stop=False, skip_group_check=True)
                    nc.tensor.matmul(dst_ap, b2_sb[l][:],
                                     hbarT[:, pr * 256:(pr + 1) * 256],
                                     start=False, stop=False,
                                     skip_group_check=True)
                    nc.tensor.matmul(dst_ap, rw_sb[l][:],
                                     hTb[:, pr * 256:(pr + 1) * 256],
                                     start=False, stop=True,
                                     skip_group_check=True)

                # next layer's ev overlaps this layer's BN AllReduce wait
                if l < L - 1:
                    ev_next = gen_ev(l + 1)

                # BN stats: global sum & sum-of-squares over nodes
                stats = statp.tile([H, 2], f32, name="stats")
                nc.vector.tensor_reduce(stats[:, 0:1], aggps[:],
                                        axis=mybir.AxisListType.X, op=ALU.add)
                trash = spool.tile([H, NS], f32, name="trash")
                nc.scalar.activation(trash[:], aggps[:], AXF.Square,
                                     accum_out=stats[:, 1:2])
                st_in = dramp.tile([H, 2], f32, name="st_in")
                nc.sync.dma_start(st_in[:], stats[:])
                st_out = dramp.tile([H, 2], f32, name="st_out",
                                    addr_space="Shared")
                nc.gpsimd.collective_compute(
                    "AllReduce", ALU.add, replica_groups=groups,
                    ins=[st_in.opt()], outs=[st_out.opt()])
                stats2 = statp.tile([H, 2], f32, name="stats2")
                nc.sync.dma_start(stats2[:], st_out[:])

                mu = statp.tile([H, 1], f32, name="mu")
                nc.scalar.mul(mu[:], stats2[:, 0:1], 1.0 / N)
                ex2 = statp.tile([H, 1], f32, name="ex2")
                nc.scalar.mul(ex2[:], stats2[:, 1:2], 1.0 / N)
                musq = statp.tile([H, 1], f32, name="musq")
                nc.vector.tensor_mul(musq[:], mu[:], mu[:])
                var = statp.tile([H, 1], f32, name="var")
                nc.vector.tensor_tensor(out=var[:], in0=ex2[:], in1=musq[:],
                                        op=ALU.subtract)
                std = statp.tile([H, 1], f32, name="std")
                nc.scalar.activation(std[:], var[:], AXF.Sqrt,
                                     bias=eps_sb[:, 0:1])
                rstd = statp.tile([H, 1], f32, name="rstd")
                nc.vector.reciprocal(rstd[:], std[:])
                scal = statp.tile([H, 1], f32, name="scal")
                nc.vector.tensor_mul(scal[:], rstd[:], bng_sb[l][:])
                mscal = statp.tile([H, 1], f32, name="mscal")
                nc.vector.tensor_mul(mscal[:], mu[:], scal[:])
                shift = statp.tile([H, 1], f32, name="shift")
                nc.vector.tensor_tensor(out=shift[:], in0=bnb_sb[l][:],
                                        in1=mscal[:], op=ALU.subtract)

                relu_o = spool.tile([H, NS], f32, name="relu_o")
                nc.scalar.activation(relu_o[:], aggps[:], AXF.Relu,
                                     bias=shift[:, 0:1], scale=scal[:, 0:1])
                nc.vector.tensor_add(hT[:], hT[:], relu_o[:])

                # write updated slice (rows, bf16); AllGather except last layer
                if l < L - 1:
                    hsl = dramp.tile([NS, H], bf16, name="hsl")
                    for j in range(NS // P):
                        pst = ps_ev.tile([P, P], f32, name="hup_ps", tag="e")
                        nc.tensor.transpose(pst[:], hT[:, j * P:(j + 1) * P],
                                            idf_sb[:])
                        hrow = spool.tile([P, H], bf16, name="hup_row")
                        nc.scalar.copy(hrow[:], pst[:])
                        nc.sync.dma_start(hsl[j * P:(j + 1) * P, :], hrow[:])
                    hfull = dramp.tile([N, H], bf16, name="hfull")
                    nc.gpsimd.collective_compute(
                        "AllGather", ALU.bypass, replica_groups=groups,
                        ins=[hsl.opt()], outs=[hfull.opt()])
                    h_rows = hfull
                else:
                    for j in range(NS // P):
                        pst = ps_ev.tile([P, P], f32, name="hup_ps", tag="e")
                        nc.tensor.transpose(pst[:], hT[:, j * P:(j + 1) * P],
                                            idf_sb[:])
                        nc.scalar.copy(hrow_last[j][:], pst[:])

            # ---- head: sharded global-mean-pool + AllReduce + MLP ----
            with tc.tile_pool(name="headp", bufs=1) as headp:
                pmat_sb = headp.tile([P, 4 * G], bf16, bufs=1)
                nc.sync.dma_start(pmat_sb[:], pmat_d[:])
                hw1_sb = headp.tile([H, H], bf16, bufs=1)
                nc.sync.dma_start(hw1_sb[:], hw1_d[:])
                hb1_sb = headp.tile([H, 1], f32, bufs=1)
                nc.sync.dma_start(hb1_sb[:], hb1_d[:])
                hw2_sb = headp.tile([H, 1], bf16, bufs=1)
                nc.sync.dma_start(hw2_sb[:], hw2_d[:])
                hb2_sb = headp.tile([1, 1], f32, bufs=1)
                nc.sync.dma_start(hb2_sb[:], hb2_d[:])

                ps_pool = ps_agg.tile([H, G], f32, name="pool_ps", tag="a")
                for j in range(NS // P):
                    nc.tensor.matmul(ps_pool[:], hrow_last[j][:],
                                     pmat_sb[:, j * G:(j + 1) * G],
                                     start=(j == 0), stop=(j == NS // P - 1))
                pool_part = headp.tile([H, G], f32, name="pool_part")
                nc.vector.tensor_copy(pool_part[:], ps_pool[:])
                pl_in = dramp.tile([H, G], f32, name="pl_in")
                nc.sync.dma_start(pl_in[:], pool_part[:])
                pl_out = dramp.tile([H, G], f32, name="pl_out",
                                    addr_space="Shared")
                nc.gpsimd.collective_compute(
                    "AllReduce", ALU.add, replica_groups=groups,
                    ins=[pl_in.opt()], outs=[pl_out.opt()])
                pool_f = headp.tile([H, G], f32, name="pool_f")
                nc.sync.dma_start(pool_f[:], pl_out[:])
                pooledT = headp.tile([H, G], bf16, name="pooledT")
                nc.vector.tensor_copy(pooledT[:], pool_f[:])

                ps_z = ps_ev.tile([H, G], f32, name="z_ps", tag="e")
                nc.tensor.matmul(ps_z[:], hw1_sb[:], pooledT[:],
                                 start=True, stop=True)
                z = headp.tile([H, G], bf16, name="z")
                nc.scalar.activation(z[:], ps_z[:], AXF.Relu,
                                     bias=hb1_sb[:, 0:1])
                ps_y = ps_ev.tile([1, G], f32, name="y_ps", tag="e")
                nc.tensor.matmul(ps_y[:], hw2_sb[:], z[:], start=True,
                                 stop=True)
                ysb = headp.tile([1, G], f32, name="ysb")
                nc.vector.tensor_scalar_add(ysb[:], ps_y[:],
                                            hb2_sb[0:1, 0:1])
                nc.sync.dma_start(y_d[:], ysb[:])

    nc.compile()
    return nc


# ----------------------------------------------------------------------------
# Entry point
# ----------------------------------------------------------------------------

def kernel(**inputs):
    inp = {k: np.asarray(v) for k, v in inputs.items()}
    cores = _preprocess(inp["edge_index"], inp["edge_attr"])

    bf = lambda a: np.ascontiguousarray(np.asarray(a, np.float32)).astype(BF16)
    f32 = lambda a: np.ascontiguousarray(np.asarray(a, np.float32))

    e1w = np.concatenate(
        [np.asarray(inp["e1_w"], np.float32),
         np.asarray(inp["e1_b"], np.float32)[:, None, :]], axis=1)  # [L,17,128]
    # e2w_stack[l][h, k*128+o] = e2_w[l][k, h*128+o]
    w2s = np.asarray(inp["e2_w"], np.float32).reshape(L, H, H, H) \
        .transpose(0, 2, 1, 3).reshape(L, H, H * H)
    b2s = np.asarray(inp["e2_b"], np.float32).reshape(L, H, H)  # [h, o]
    xa = np.concatenate([np.asarray(inp["x"], np.float32).T,
                         np.ones((1, N), np.float32)], 0)  # [65, N]
    nw = np.concatenate([np.asarray(inp["node_w"], np.float32),
                         np.asarray(inp["node_b"], np.float32)[None, :]], 0)

    batch = np.asarray(inp["batch"], np.int64)
    cnt = np.bincount(batch, minlength=G).astype(np.float32)
    Pm = np.zeros((N, G), np.float32)
    Pm[np.arange(N), batch] = 1.0 / np.maximum(cnt, 1.0)[batch]

    shared = dict(
        e1w=bf(e1w), w2s=bf(w2s), b2s=bf(b2s), rw=bf(inp["root_w"]),
        bng=f32(inp["bn_g"])[:, :, None], bnb=f32(inp["bn_b"])[:, :, None],
        nw=bf(nw), hw1=bf(inp["head_w1"]), hb1=f32(inp["head_b1"])[:, None],
        hw2=bf(inp["head_w2"]), hb2=f32(inp["head_b2"])[None, :],
        idf=np.eye(P, dtype=np.float32),
    )

    in_maps = []
    for c in range(NC):
        cd = cores[c]
        m = dict(shared)
        m["ea"] = bf(cd["eaT"])
        m["srcg"] = np.ascontiguousarray(cd["srcg"])
        m["mscale"] = f32(cd["mscale"])
        m["smat"] = bf(cd["smat"])
        m["xs"] = bf(xa[:, c * NS:(c + 1) * NS])
        pm = np.zeros((P, 4 * G), np.float32)
        for j in range(NS // P):
            pm[:, j * G:(j + 1) * G] = Pm[c * NS + j * P: c * NS + (j + 1) * P]
        m["pmat"] = bf(pm)
        in_maps.append(m)

    nc = _build()
    import os
    trace = os.environ.get("KERNEL_TRACE", "0") == "1"
    res = run_bass_kernel_spmd(nc, in_maps, list(range(NC)), trace=trace)
    if trace and res.exec_time_ns is not None:
        print(f"HW exec time: {res.exec_time_ns} ns")
    y = np.asarray(res.results[0]["y"], np.float32).reshape(G)
    return y


# revision 16
# speedup vs baseline: 1.2982x; 1.2982x over previous
"""DMPNN (NNConv edge-network message passing) Trainium2 kernel, 8-core SPMD.

Algorithm: instead of materializing per-edge [H,H] weights (the reference's
W_e = relu(ea@e1)@e2, then msg_e = h_src W_e, scatter-mean), contract edges
into dst nodes FIRST via per-node outer products:

  C[n, (k,h)] = sum_{e->n} ev[e,k] * h_src[e,h] / deg_n      (tiny PE matmuls)
  aggT[o, n]  = sum_{k,h} e2w[k,h,o] * C[n,(k,h)]            (dense PE matmul)
              + hbar_n @ e2b + h @ root_w                     (bias + root)

This cuts the big contraction from E to N columns and avoids any E x 16384
intermediate. Per-node matmuls use 32-row PE array tiling: each dst node
group of 4 ("quadrant", 32 slots) does ONE [32]x[32,512] matmul with
color-masked ev (mask zeroes other nodes' rows; applied free during the
relu eviction).

Sharding: dst-node range per core (512 nodes). Per layer: AllReduce of BN
stats + AllGather of updated node features.
"""

import numpy as np
import ml_dtypes

import concourse.bass as bass
import concourse.tile as tile
import concourse.mybir as mybir
from concourse import bacc
from concourse.bass import IndirectOffsetOnAxis
from concourse.bass_utils import run_bass_kernel_spmd

BF16 = ml_dtypes.bfloat16

N, E, F_NODE, F_EDGE, H, L, G = 4096, 12288, 64, 16, 128, 4, 256
NC = 8
NS = N // NC            # nodes per core (512)
P = 128
T = 32                  # slot tiles per core (32 x 128 slots)
NQ = 4 * T              # quadrants (one per 4 nodes)
NCHUNK = 4              # agg chunks of 128 nodes
BN_EPS = 1e-5
AXF = mybir.ActivationFunctionType
ALU = mybir.AluOpType


# ----------------------------------------------------------------------------
# Host preprocessing
# ----------------------------------------------------------------------------

def _preprocess(edge_index, edge_attr):
    src = np.asarray(edge_index[0], dtype=np.int64)
    dst = np.asarray(edge_index[1], dtype=np.int64)
    ea = np.asarray(edge_attr, dtype=np.float32)
    deg = np.bincount(dst, minlength=N).astype(np.float32)
    inv_deg = np.where(deg > 0, 1.0 / np.maximum(deg, 1.0), 0.0).astype(np.float32)

    # edges grouped by dst
    order = np.argsort(dst, kind="stable")
    starts = np.searchsorted(dst[order], np.arange(N))
    ends = np.searchsorted(dst[order], np.arange(N), side="right")

    cores = []
    for c in range(NC):
        eaT = np.zeros((17, T * P), np.float32)
        srcg = np.zeros((P, T), np.int32)
        mscale = np.zeros((P, T * 4), np.float32)
        smat = np.zeros((P, T * 16), np.float32)
        for t in range(T):
            for q in range(4):
                p0 = 32 * q
                fill = 0
                for cidx in range(4):
                    n_local = 16 * t + 4 * q + cidx
                    n_glob = c * NS + n_local
                    es = order[starts[n_glob]:ends[n_glob]]
                    k = len(es)
                    assert fill + k <= 32, (c, t, q, fill, k)
                    sl = slice(p0 + fill, p0 + fill + k)
                    eaT[:F_EDGE, t * P + p0 + fill: t * P + p0 + fill + k] = ea[es].T
                    eaT[F_EDGE, t * P + p0 + fill: t * P + p0 + fill + k] = 1.0
                    srcg[sl, t] = src[es]
                    mscale[sl, t * 4 + cidx] = inv_deg[n_glob]
                    smat[sl, t * 16 + 4 * q + cidx] = inv_deg[n_glob]
                    fill += k
        cores.append(dict(eaT=eaT, srcg=srcg, mscale=mscale, smat=smat))
    return cores


# ----------------------------------------------------------------------------
# Device program
# ----------------------------------------------------------------------------

def _build():
    f32 = mybir.dt.float32
    bf16 = mybir.dt.bfloat16
    fp8 = mybir.dt.float8e4
    i32 = mybir.dt.int32
    nc = bacc.Bacc("TRN2", target_bir_lowering=False, debug=False, num_devices=NC)

    def din(name, shape, dt=bf16):
        return nc.dram_tensor(name, shape, dt, kind="ExternalInput")

    ea_d = din("ea", [17, T * P])
    srcg_d = din("srcg", [P, T], i32)
    mscale_d = din("mscale", [P, T * 4], f32)
    smat_d = din("smat", [P, T * 16])
    e1w_d = din("e1w", [L, 17, H])
    w2s_d = din("w2s", [L, H, H * H])      # e2w_stack[l][h, k*128+o]
    b2s_d = din("b2s", [L, H, H])          # e2_b as [h, o]
    rw_d = din("rw", [L, H, H])            # root_w as [h, o]
    bng_d = din("bng", [L, H, 1], f32)
    bnb_d = din("bnb", [L, H, 1], f32)
    xs_d = din("xs", [65, NS])
    nw_d = din("nw", [65, H])
    pmat_d = din("pmat", [P, 4 * G])
    hw1_d = din("hw1", [H, H])
    hb1_d = din("hb1", [H, 1], f32)
    hw2_d = din("hw2", [H, 1])
    hb2_d = din("hb2", [1, 1], f32)
    idf_d = din("idf", [P, P], f32)
    y_d = nc.dram_tensor("y", [1, G], f32, kind="ExternalOutput")

    groups = [list(range(NC))]

    with tile.TileContext(nc) as tc:
        with tc.tile_pool(name="const", bufs=1) as const, \
             tc.tile_pool(name="persist", bufs=1) as persist, \
             tc.tile_pool(name="w2pool", bufs=1) as w2pool, \
             tc.tile_pool(name="cpool", bufs=2) as cpool, \
             tc.tile_pool(name="evpool", bufs=2) as evpool, \
             tc.tile_pool(name="hspool", bufs=1) as hspool, \
             tc.tile_pool(name="spool", bufs=2) as spool, \
             tc.tile_pool(name="stat", bufs=2) as statp, \
             tc.tile_pool(name="psc", bufs=2, space="PSUM") as ps_c, \
             tc.tile_pool(name="psagg", bufs=1, space="PSUM") as ps_agg, \
             tc.tile_pool(name="pshb", bufs=1, space="PSUM") as ps_hb, \
             tc.tile_pool(name="psev", bufs=2, space="PSUM") as ps_ev, \
             tc.tile_pool(name="dramp", bufs=2, space="DRAM") as dramp:

            # ---- persistent constants ----
            ea_sb = const.tile([17, T * P], bf16)
            nc.sync.dma_start(ea_sb[:], ea_d[:])
            srcg_sb = const.tile([P, T], i32)
            nc.sync.dma_start(srcg_sb[:], srcg_d[:])
            mscale_sb = const.tile([P, T * 4], f32)
            nc.sync.dma_start(mscale_sb[:], mscale_d[:])
            smat_sb = const.tile([P, T * 16], bf16)
            nc.sync.dma_start(smat_sb[:], smat_d[:])
            xs_sb = const.tile([65, NS], bf16)
            nc.sync.dma_start(xs_sb[:], xs_d[:])
            nw_sb = const.tile([65, H], bf16)
            nc.sync.dma_start(nw_sb[:], nw_d[:])
            idf_sb = const.tile([P, P], f32)
            nc.sync.dma_start(idf_sb[:], idf_d[:])
            e1w_sb, b2_sb, rw_sb, bng_sb, bnb_sb = [], [], [], [], []
            for l in range(L):
                t_ = const.tile([17, H], bf16, name=f"e1w_{l}")
                nc.sync.dma_start(t_[:], e1w_d[l])
                e1w_sb.append(t_)
                t_ = const.tile([H, H], bf16, name=f"b2_{l}")
                nc.sync.dma_start(t_[:], b2s_d[l])
                b2_sb.append(t_)
                t_ = const.tile([H, H], bf16, name=f"rw_{l}")
                nc.sync.dma_start(t_[:], rw_d[l])
                rw_sb.append(t_)
                t_ = const.tile([H, 1], f32, name=f"bng_{l}")
                nc.sync.dma_start(t_[:], bng_d[l])
                bng_sb.append(t_)
                t_ = const.tile([H, 1], f32, name=f"bnb_{l}")
                nc.sync.dma_start(t_[:], bnb_d[l])
                bnb_sb.append(t_)
            eps_sb = const.tile([H, 1], f32)
            nc.vector.memset(eps_sb[:], BN_EPS)

            hT = persist.tile([H, NS], f32)      # own node features, [h, n]
            hrow_last = [persist.tile([P, H], bf16, name=f"hrl_{j}")
                         for j in range(4)]

            # ---- node encoder (own slice only) + AllGather ----
            hsl0 = dramp.tile([NS, H], bf16, name="hsl0", bufs=1)
            with tc.tile_pool(name="encp", bufs=2) as encp:
                for j in range(NS // P):
                    ps = ps_ev.tile([P, H], f32, name="enc_ps", tag="e")
                    nc.tensor.matmul(ps[:], xs_sb[:, j * P:(j + 1) * P], nw_sb[:],
                                     start=True, stop=True)
                    tmp = encp.tile([P, H], f32, name="enc_tmp")
                    nc.vector.tensor_copy(tmp[:], ps[:])
                    hrow = encp.tile([P, H], bf16, name="enc_row")
                    nc.scalar.copy(hrow[:], ps[:])
                    nc.sync.dma_start(hsl0[j * P:(j + 1) * P, :], hrow[:])
                    ps2 = ps_ev.tile([P, P], f32, name="enc_ps2", tag="e")
                    nc.tensor.transpose(ps2[:], tmp[:], idf_sb[:])
                    nc.scalar.copy(hT[:, j * P:(j + 1) * P], ps2[:])
            hfull0 = dramp.tile([N, H], bf16, name="hfull0", bufs=1)
            nc.gpsimd.collective_compute(
                "AllGather", ALU.bypass, replica_groups=groups,
                ins=[hsl0.opt()], outs=[hfull0.opt()])
            h_rows = hfull0

            # ev generation + color-masked eviction (scaled by 1/deg).
            # Depends only on edge_attr, so layer l+1's ev runs during
            # layer l's BN AllReduce wait.
            def gen_ev(l):
                # ev_all tile-block columns are (k, color)-interleaved so the
                # C-build psum comes out k-major.
                ev_all = evpool.tile([P, T * 512], bf16, name="ev_all")
                evv = ev_all[:].rearrange("p (t k c) -> p t k c", t=T, c=4)
                for t in range(T):
                    evps = ps_ev.tile([P, H], f32, name="evps", tag="e")
                    nc.tensor.matmul(evps[:], ea_sb[:17, t * P:(t + 1) * P],
                                     e1w_sb[l][:], start=True, stop=True)
                    for cidx in range(4):
                        dst_ap = evv[:, t, :, cidx]
                        sc = mscale_sb[:, t * 4 + cidx: t * 4 + cidx + 1]
                        if cidx % 2 == 0:
                            nc.vector.tensor_scalar(
                                out=dst_ap, in0=evps[:], scalar1=sc,
                                scalar2=0.0, op0=ALU.mult, op1=ALU.max)
                        else:
                            nc.scalar.activation(dst_ap, evps[:], AXF.Relu,
                                                 scale=sc)
                return ev_all

            ev_next = gen_ev(0)

            # ---- layers ----
            for l in range(L):
                ev_all = ev_next
                w2_sb = w2pool.tile([H, H * H], bf16, name="w2")
                nc.sync.dma_start(w2_sb[:], w2s_d[l])

                hTb = spool.tile([H, NS], bf16, name="hTb")
                nc.vector.tensor_copy(hTb[:], hT[:])

                # gather h_src rows for all slots
                hs_all = hspool.tile([P, T * H], bf16, name="hs_all")
                for t in range(T):
                    nc.gpsimd.indirect_dma_start(
                        out=hs_all[:, t * H:(t + 1) * H], out_offset=None,
                        in_=h_rows[:],
                        in_offset=IndirectOffsetOnAxis(
                            ap=srcg_sb[:, t:t + 1], axis=0))

                # hbar[h, n] = scatter-mean of h_src (for the e2_b term)
                hbps = ps_hb.tile([H, NS], f32, name="hbps")
                for t in range(T):
                    nc.tensor.matmul(hbps[:, t * 16:(t + 1) * 16],
                                     hs_all[:, t * H:(t + 1) * H],
                                     smat_sb[:, t * 16:(t + 1) * 16],
                                     start=True, stop=True,
                                     skip_group_check=True)
                hbarT = spool.tile([H, NS], bf16, name="hbarT")
                nc.vector.tensor_copy(hbarT[:], hbps[:])

                # Per 256-node pair: C-build (one [32]x[32,512] matmul per
                # quadrant, two quadrants per 2-bank psum tile; psum cols are
                # (k, color) so C evicts k-major), then agg with 256
                # CONTIGUOUS moving columns per w2 weight-load.
                aggps = ps_agg.tile([H, NS], f32, name="aggps", tag="a")
                ei = 0
                for pr in range(NCHUNK // 2):
                    cc = cpool.tile([H, 256 * H], fp8, name="cc")
                    ccv = cc[:].rearrange("p (k n) -> p k n", k=H)
                    for tt in range(16):
                        t = pr * 16 + tt
                        for qp in range(2):
                            cps = ps_c.tile([H, 1024], f32, name="cps")
                            for qi in range(2):
                                q = qp * 2 + qi
                                nc.tensor.matmul(
                                    cps[:, qi * 512:(qi + 1) * 512],
                                    hs_all[32 * q:32 * (q + 1),
                                           t * H:(t + 1) * H],
                                    ev_all[32 * q:32 * (q + 1),
                                           t * 512:(t + 1) * 512],
                                    start=True, stop=True,
                                    tile_position=(32 * q, 0),
                                    skip_group_check=True)
                            for qi in range(2):
                                nb = tt * 16 + (qp * 2 + qi) * 4
                                src = cps[:, qi * 512:(qi + 1) * 512]
                                dst = ccv[:, :, nb:nb + 4]
                                if ei % 2 == 0:
                                    nc.vector.tensor_copy(dst, src)
                                else:
                                    nc.scalar.copy(dst, src)
                                ei += 1
                    dst_ap = aggps[:, pr * 256:(pr + 1) * 256]
                    for k in range(H):
                        nc.tensor.matmul(dst_ap, w2_sb[:, k * P:(k + 1) * P],
                                         cc[:, k * 256:(k + 1) * 256],
                                         start=(k == 0),
                                         stop=False, skip_group_check=True)
                    nc.tensor.matmul(dst_ap, b2_sb[l][:],
                                     hbarT[:, pr * 256:(pr + 1) * 256],
                                     start=False, stop=False,
                                     skip_group_check=True)
                    nc.tensor.matmul(dst_ap, rw_sb[l][:],
                                     hTb[:, pr * 256:(pr + 1) * 256],
                                     start=False, stop=True,
                                     skip_group_check=True)

                # next layer's ev overlaps this layer's BN AllReduce wait
                if l < L - 1:
                    ev_next = gen_ev(l + 1)

                # BN stats: global sum & sum-of-squares over nodes
                stats = statp.tile([H, 2], f32, name="stats")
                nc.vector.tensor_reduce(stats[:, 0:1], aggps[:],
                                        axis=mybir.AxisListType.X, op=ALU.add)
                trash = spool.tile([H, NS], f32, name="trash")
                nc.scalar.activation(trash[:], aggps[:], AXF.Square,
                                     accum_out=stats[:, 1:2])
                st_in = dramp.tile([H, 2], f32, name="st_in")
                nc.sync.dma_start(st_in[:], stats[:])
                st_out = dramp.tile([H, 2], f32, name="st_out",
                                    addr_space="Shared")
                nc.gpsimd.collective_compute(
                    "AllReduce", ALU.add, replica_groups=groups,
                    ins=[st_in.opt()], outs=[st_out.opt()])
                stats2 = statp.tile([H, 2], f32, name="stats2")
                nc.sync.dma_start(stats2[:], st_out[:])

                mu = statp.tile([H, 1], f32, name="mu")
                nc.scalar.mul(mu[:], stats2[:, 0:1], 1.0 / N)
                ex2 = statp.tile([H, 1], f32, name="ex2")
                nc.scalar.mul(ex2[:], stats2[:, 1:2], 1.0 / N)
                musq = statp.tile([H, 1], f32, name="musq")
                nc.vector.tensor_mul(musq[:], mu[:], mu[:])
                var = statp.tile([H, 1], f32, name="var")
                nc.vector.tensor_tensor(out=var[:], in0=ex2[:], in1=musq[:],
                                        op=ALU.subtract)
                std = statp.tile([H, 1], f32, name="std")
                nc.scalar.activation(std[:], var[:], AXF.Sqrt,
                                     bias=eps_sb[:, 0:1])
                rstd = statp.tile([H, 1], f32, name="rstd")
                nc.vector.reciprocal(rstd[:], std[:])
                scal = statp.tile([H, 1], f32, name="scal")
                nc.vector.tensor_mul(scal[:], rstd[:], bng_sb[l][:])
                mscal = statp.tile([H, 1], f32, name="mscal")
                nc.vector.tensor_mul(mscal[:], mu[:], scal[:])
                shift = statp.tile([H, 1], f32, name="shift")
                nc.vector.tensor_tensor(out=shift[:], in0=bnb_sb[l][:],
                                        in1=mscal[:], op=ALU.subtract)

                relu_o = spool.tile([H, NS], f32, name="relu_o")
                nc.scalar.activation(relu_o[:], aggps[:], AXF.Relu,
                                     bias=shift[:, 0:1], scale=scal[:, 0:1])
                nc.vector.tensor_add(hT[:], hT[:], relu_o[:])

                # write updated slice (rows, bf16); AllGather except last layer
                if l < L - 1:
                    hsl = dramp.tile([NS, H], bf16, name="hsl")
                    for j in range(NS // P):
                        pst = ps_ev.tile([P, P], f32, name="hup_ps", tag="e")
                        nc.tensor.transpose(pst[:], hT[:, j * P:(j + 1) * P],
                                            idf_sb[:])
                        hrow = spool.tile([P, H], bf16, name="hup_row")
                        nc.scalar.copy(hrow[:], pst[:])
                        nc.sync.dma_start(hsl[j * P:(j + 1) * P, :], hrow[:])
                    hfull = dramp.tile([N, H], bf16, name="hfull")
                    nc.gpsimd.collective_compute(
                        "AllGather", ALU.bypass, replica_groups=groups,
                        ins=[hsl.opt()], outs=[hfull.opt()])
                    h_rows = hfull
                else:
                    for j in range(NS // P):
                        pst = ps_ev.tile([P, P], f32, name="hup_ps", tag="e")
                        nc.tensor.transpose(pst[:], hT[:, j * P:(j + 1) * P],
                                            idf_sb[:])
                        nc.scalar.copy(hrow_last[j][:], pst[:])

            # ---- head: sharded global-mean-pool + AllReduce + MLP ----
            with tc.tile_pool(name="headp", bufs=1) as headp:
                pmat_sb = headp.tile([P, 4 * G], bf16, bufs=1)
                nc.sync.dma_start(pmat_sb[:], pmat_d[:])
                hw1_sb = headp.tile([H, H], bf16, bufs=1)
                nc.sync.dma_start(hw1_sb[:], hw1_d[:])
                hb1_sb = headp.tile([H, 1], f32, bufs=1)
                nc.sync.dma_start(hb1_sb[:], hb1_d[:])
                hw2_sb = headp.tile([H, 1], bf16, bufs=1)
                nc.sync.dma_start(hw2_sb[:], hw2_d[:])
                hb2_sb = headp.tile([1, 1], f32, bufs=1)
                nc.sync.dma_start(hb2_sb[:], hb2_d[:])

                ps_pool = ps_agg.tile([H, G], f32, name="pool_ps", tag="a")
                for j in range(NS // P):
                    nc.tensor.matmul(ps_pool[:], hrow_last[j][:],
                                     pmat_sb[:, j * G:(j + 1) * G],
                                     start=(j == 0), stop=(j == NS // P - 1))
                pool_part = headp.tile([H, G], f32, name="pool_part")
                nc.vector.tensor_copy(pool_part[:], ps_pool[:])
                pl_in = dramp.tile([H, G], f32, name="pl_in")
                nc.sync.dma_start(pl_in[:], pool_part[:])
                pl_out = dramp.tile([H, G], f32, name="pl_out",
                                    addr_space="Shared")
                nc.gpsimd.collective_compute(
                    "AllReduce", ALU.add, replica_groups=groups,
                    ins=[pl_in.opt()], outs=[pl_out.opt()])
                pool_f = headp.tile([H, G], f32, name="pool_f")
                nc.sync.dma_start(pool_f[:], pl_out[:])
                pooledT = headp.tile([H, G], bf16, name="pooledT")
                nc.vector.tensor_copy(pooledT[:], pool_f[:])

                ps_z = ps_ev.tile([H, G], f32, name="z_ps", tag="e")
                nc.tensor.matmul(ps_z[:], hw1_sb[:], pooledT[:],
                                 start=True, stop=True)
                z = headp.tile([H, G], bf16, name="z")
                nc.scalar.activation(z[:], ps_z[:], AXF.Relu,
                                     bias=hb1_sb[:, 0:1])
                ps_y = ps_ev.tile([1, G], f32, name="y_ps", tag="e")
                nc.tensor.matmul(ps_y[:], hw2_sb[:], z[:], start=True,
                                 stop=True)
                ysb = headp.tile([1, G], f32, name="ysb")
                nc.vector.tensor_scalar_add(ysb[:], ps_y[:],
                                            hb2_sb[0:1, 0:1])
                nc.sync.dma_start(y_d[:], ysb[:])

    nc.compile()
    return nc


# ----------------------------------------------------------------------------
# Entry point
# ----------------------------------------------------------------------------

def kernel(**inputs):
    inp = {k: np.asarray(v) for k, v in inputs.items()}
    cores = _preprocess(inp["edge_index"], inp["edge_attr"])

    bf = lambda a: np.ascontiguousarray(np.asarray(a, np.float32)).astype(BF16)
    f32 = lambda a: np.ascontiguousarray(np.asarray(a, np.float32))

    e1w = np.concatenate(
        [np.asarray(inp["e1_w"], np.float32),
         np.asarray(inp["e1_b"], np.float32)[:, None, :]], axis=1)  # [L,17,128]
    # e2w_stack[l][h, k*128+o] = e2_w[l][k, h*128+o]
    w2s = np.asarray(inp["e2_w"], np.float32).reshape(L, H, H, H) \
        .transpose(0, 2, 1, 3).reshape(L, H, H * H)
    b2s = np.asarray(inp["e2_b"], np.float32).reshape(L, H, H)  # [h, o]
    xa = np.concatenate([np.asarray(inp["x"], np.float32).T,
                         np.ones((1, N), np.float32)], 0)  # [65, N]
    nw = np.concatenate([np.asarray(inp["node_w"], np.float32),
                         np.asarray(inp["node_b"], np.float32)[None, :]], 0)

    batch = np.asarray(inp["batch"], np.int64)
    cnt = np.bincount(batch, minlength=G).astype(np.float32)
    Pm = np.zeros((N, G), np.float32)
    Pm[np.arange(N), batch] = 1.0 / np.maximum(cnt, 1.0)[batch]

    shared = dict(
        e1w=bf(e1w), w2s=bf(w2s), b2s=bf(b2s), rw=bf(inp["root_w"]),
        bng=f32(inp["bn_g"])[:, :, None], bnb=f32(inp["bn_b"])[:, :, None],
        nw=bf(nw), hw1=bf(inp["head_w1"]), hb1=f32(inp["head_b1"])[:, None],
        hw2=bf(inp["head_w2"]), hb2=f32(inp["head_b2"])[None, :],
        idf=np.eye(P, dtype=np.float32),
    )

    in_maps = []
    for c in range(NC):
        cd = cores[c]
        m = dict(shared)
        m["ea"] = bf(cd["eaT"])
        m["srcg"] = np.ascontiguousarray(cd["srcg"])
        m["mscale"] = f32(cd["mscale"])
        m["smat"] = bf(cd["smat"])
        m["xs"] = bf(xa[:, c * NS:(c + 1) * NS])
        pm = np.zeros((P, 4 * G), np.float32)
        for j in range(NS // P):
            pm[:, j * G:(j + 1) * G] = Pm[c * NS + j * P: c * NS + (j + 1) * P]
        m["pmat"] = bf(pm)
        in_maps.append(m)

    nc = _build()
    import os
    trace = os.environ.get("KERNEL_TRACE", "0") == "1"
    res = run_bass_kernel_spmd(nc, in_maps, list(range(NC)), trace=trace)
    if trace and res.exec_time_ns is not None:
        print(f"HW exec time: {res.exec_time_ns} ns")
    y = np.asarray(res.results[0]["y"], np.float32).reshape(G)
    return y


# revision 21
# speedup vs baseline: 1.3154x; 1.0132x over previous
"""DMPNN (NNConv edge-network message passing) Trainium2 kernel, 8-core SPMD.

Algorithm: instead of materializing per-edge [H,H] weights (the reference's
W_e = relu(ea@e1)@e2, then msg_e = h_src W_e, scatter-mean), contract edges
into dst nodes FIRST via per-node outer products:

  C[n, (k,h)] = sum_{e->n} ev[e,k] * h_src[e,h] / deg_n      (tiny PE matmuls)
  aggT[o, n]  = sum_{k,h} e2w[k,h,o] * C[n,(k,h)]            (dense PE matmul)
              + hbar_n @ e2b + h @ root_w                     (bias + root)

This cuts the big contraction from E to N columns and avoids any E x 16384
intermediate. Per-node matmuls use 32-row PE array tiling: each dst node
group of 4 ("quadrant", 32 slots) does ONE [32]x[32,512] matmul with
color-masked ev (mask zeroes other nodes' rows; applied free during the
relu eviction).

Sharding: dst-node range per core (512 nodes). Per layer: AllReduce of BN
stats + AllGather of updated node features.
"""

import numpy as np
import ml_dtypes

import concourse.bass as bass
import concourse.tile as tile
import concourse.mybir as mybir
from concourse import bacc
from concourse.bass import IndirectOffsetOnAxis
from concourse.bass_utils import run_bass_kernel_spmd

BF16 = ml_dtypes.bfloat16

N, E, F_NODE, F_EDGE, H, L, G = 4096, 12288, 64, 16, 128, 4, 256
NC = 8
NS = N // NC            # nodes per core (512)
P = 128
T = 32                  # slot tiles per core (32 x 128 slots)
NQ = 4 * T              # quadrants (one per 4 nodes)
NCHUNK = 4              # agg chunks of 128 nodes
BN_EPS = 1e-5
AXF = mybir.ActivationFunctionType
ALU = mybir.AluOpType


# ----------------------------------------------------------------------------
# Host preprocessing
# ----------------------------------------------------------------------------

def _preprocess(edge_index, edge_attr):
    src = np.asarray(edge_index[0], dtype=np.int64)
    dst = np.asarray(edge_index[1], dtype=np.int64)
    ea = np.asarray(edge_attr, dtype=np.float32)
    deg = np.bincount(dst, minlength=N).astype(np.float32)
    inv_deg = np.where(deg > 0, 1.0 / np.maximum(deg, 1.0), 0.0).astype(np.float32)

    # edges grouped by dst
    order = np.argsort(dst, kind="stable")
    starts = np.searchsorted(dst[order], np.arange(N))
    ends = np.searchsorted(dst[order], np.arange(N), side="right")

    cores = []
    for c in range(NC):
        eaT = np.zeros((17, T * P), np.float32)
        srcg = np.zeros((P, T), np.int32)
        mscale = np.zeros((P, T * 4), np.float32)
        smat = np.zeros((P, T * 16), np.float32)
        for t in range(T):
            for q in range(4):
                p0 = 32 * q
                fill = 0
                for cidx in range(4):
                    n_local = 16 * t + 4 * q + cidx
                    n_glob = c * NS + n_local
                    es = order[starts[n_glob]:ends[n_glob]]
                    k = len(es)
                    assert fill + k <= 32, (c, t, q, fill, k)
                    sl = slice(p0 + fill, p0 + fill + k)
                    eaT[:F_EDGE, t * P + p0 + fill: t * P + p0 + fill + k] = ea[es].T
                    eaT[F_EDGE, t * P + p0 + fill: t * P + p0 + fill + k] = 1.0
                    srcg[sl, t] = src[es]
                    mscale[sl, t * 4 + cidx] = inv_deg[n_glob]
                    smat[sl, t * 16 + 4 * q + cidx] = inv_deg[n_glob]
                    fill += k
        cores.append(dict(eaT=eaT, srcg=srcg, mscale=mscale, smat=smat))
    return cores


# ----------------------------------------------------------------------------
# Device program
# ----------------------------------------------------------------------------

def _build():
    f32 = mybir.dt.float32
    bf16 = mybir.dt.bfloat16
    fp8 = mybir.dt.float8e4
    i32 = mybir.dt.int32
    nc = bacc.Bacc("TRN2", target_bir_lowering=False, debug=False, num_devices=NC)

    def din(name, shape, dt=bf16):
        return nc.dram_tensor(name, shape, dt, kind="ExternalInput")

    ea_d = din("ea", [17, T * P])
    srcg_d = din("srcg", [P, T], i32)
    mscale_d = din("mscale", [P, T * 4], f32)
    smat_d = din("smat", [P, T * 16])
    e1w_d = din("e1w", [L, 17, H])
    w2s_d = din("w2s", [L, H, H * H])      # e2w_stack[l][h, k*128+o]
    b2s_d = din("b2s", [L, H, H])          # e2_b as [h, o]
    rw_d = din("rw", [L, H, H])            # root_w as [h, o]
    bng_d = din("bng", [L, H, 1], f32)
    bnb_d = din("bnb", [L, H, 1], f32)
    xs_d = din("xs", [65, NS])
    nw_d = din("nw", [65, H])
    pmat_d = din("pmat", [P, 4 * G])
    hw1_d = din("hw1", [H, H])
    hb1_d = din("hb1", [H, 1], f32)
    hw2_d = din("hw2", [H, 1])
    hb2_d = din("hb2", [1, 1], f32)
    idf_d = din("idf", [P, P], f32)
    y_d = nc.dram_tensor("y", [1, G], f32, kind="ExternalOutput")

    groups = [list(range(NC))]

    with tile.TileContext(nc) as tc:
        with tc.tile_pool(name="const", bufs=1) as const, \
             tc.tile_pool(name="persist", bufs=1) as persist, \
             tc.tile_pool(name="w2pool", bufs=1) as w2pool, \
             tc.tile_pool(name="cpool", bufs=2) as cpool, \
             tc.tile_pool(name="evpool", bufs=2) as evpool, \
             tc.tile_pool(name="hspool", bufs=1) as hspool, \
             tc.tile_pool(name="spool", bufs=2) as spool, \
             tc.tile_pool(name="stat", bufs=2) as statp, \
             tc.tile_pool(name="psc", bufs=2, space="PSUM") as ps_c, \
             tc.tile_pool(name="psagg", bufs=1, space="PSUM") as ps_agg, \
             tc.tile_pool(name="pshb", bufs=1, space="PSUM") as ps_hb, \
             tc.tile_pool(name="psev", bufs=2, space="PSUM") as ps_ev, \
             tc.tile_pool(name="dramp", bufs=2, space="DRAM") as dramp:

            # ---- persistent constants ----
            ea_sb = const.tile([17, T * P], bf16)
            nc.sync.dma_start(ea_sb[:], ea_d[:])
            srcg_sb = const.tile([P, T], i32)
            nc.sync.dma_start(srcg_sb[:], srcg_d[:])
            mscale_sb = const.tile([P, T * 4], f32)
            nc.sync.dma_start(mscale_sb[:], mscale_d[:])
            smat_sb = const.tile([P, T * 16], bf16)
            nc.sync.dma_start(smat_sb[:], smat_d[:])
            xs_sb = const.tile([65, NS], bf16)
            nc.sync.dma_start(xs_sb[:], xs_d[:])
            nw_sb = const.tile([65, H], bf16)
            nc.sync.dma_start(nw_sb[:], nw_d[:])
            idf_sb = const.tile([P, P], f32)
            nc.sync.dma_start(idf_sb[:], idf_d[:])
            e1w_sb, b2_sb, rw_sb, bng_sb, bnb_sb = [], [], [], [], []
            for l in range(L):
                t_ = const.tile([17, H], bf16, name=f"e1w_{l}")
                nc.sync.dma_start(t_[:], e1w_d[l])
                e1w_sb.append(t_)
                t_ = const.tile([H, H], bf16, name=f"b2_{l}")
                nc.sync.dma_start(t_[:], b2s_d[l])
                b2_sb.append(t_)
                t_ = const.tile([H, H], bf16, name=f"rw_{l}")
                nc.sync.dma_start(t_[:], rw_d[l])
                rw_sb.append(t_)
                t_ = const.tile([H, 1], f32, name=f"bng_{l}")
                nc.sync.dma_start(t_[:], bng_d[l])
                bng_sb.append(t_)
                t_ = const.tile([H, 1], f32, name=f"bnb_{l}")
                nc.sync.dma_start(t_[:], bnb_d[l])
                bnb_sb.append(t_)
            eps_sb = const.tile([H, 1], f32)
            nc.vector.memset(eps_sb[:], BN_EPS)

            hT = persist.tile([H, NS], f32)      # own node features, [h, n]
            hrow_last = [persist.tile([P, H], bf16, name=f"hrl_{j}")
                         for j in range(4)]

            # ---- node encoder (own slice only) + AllGather ----
            hsl0 = dramp.tile([NS, H], bf16, name="hsl0", bufs=1)
            with tc.tile_pool(name="encp", bufs=2) as encp:
                for j in range(NS // P):
                    ps = ps_ev.tile([P, H], f32, name="enc_ps", tag="e")
                    nc.tensor.matmul(ps[:], xs_sb[:, j * P:(j + 1) * P], nw_sb[:],
                                     start=True, stop=True)
                    tmp = encp.tile([P, H], f32, name="enc_tmp")
                    nc.vector.tensor_copy(tmp[:], ps[:])
                    hrow = encp.tile([P, H], bf16, name="enc_row")
                    nc.scalar.copy(hrow[:], ps[:])
                    nc.sync.dma_start(hsl0[j * P:(j + 1) * P, :], hrow[:])
                    ps2 = ps_ev.tile([P, P], f32, name="enc_ps2", tag="e")
                    nc.tensor.transpose(ps2[:], tmp[:], idf_sb[:])
                    nc.scalar.copy(hT[:, j * P:(j + 1) * P], ps2[:])
            hfull0 = dramp.tile([N, H], bf16, name="hfull0", bufs=1)
            nc.gpsimd.collective_compute(
                "AllGather", ALU.bypass, replica_groups=groups,
                ins=[hsl0.opt()], outs=[hfull0.opt()])
            h_rows = hfull0

            # ev generation + color-masked eviction (scaled by 1/deg).
            # Depends only on edge_attr, so layer l+1's ev runs during
            # layer l's BN AllReduce wait.
            def gen_ev(l):
                # ev_all tile-block columns are (k, color)-interleaved so the
                # C-build psum comes out k-major.
                ev_all = evpool.tile([P, T * 512], bf16, name="ev_all")
                for t in range(T):
                    evps = ps_ev.tile([P, H], f32, name="evps", tag="e")
                    nc.tensor.matmul(evps[:], ea_sb[:17, t * P:(t + 1) * P],
                                     e1w_sb[l][:], start=True, stop=True)
                    for cidx in range(4):
                        dst_ap = ev_all[:, t * 512 + cidx * P:
                                        t * 512 + (cidx + 1) * P]
                        sc = mscale_sb[:, t * 4 + cidx: t * 4 + cidx + 1]
                        if cidx % 2 == 0:
                            nc.vector.tensor_scalar(
                                out=dst_ap, in0=evps[:], scalar1=sc,
                                scalar2=0.0, op0=ALU.mult, op1=ALU.max)
                        else:
                            nc.scalar.activation(dst_ap, evps[:], AXF.Relu,
                                                 scale=sc)
                return ev_all

            ev_next = gen_ev(0)

            # ---- layers ----
            for l in range(L):
                ev_all = ev_next
                w2_sb = w2pool.tile([H, H * H], bf16, name="w2")
                nc.sync.dma_start(w2_sb[:], w2s_d[l])

                hTb = spool.tile([H, NS], bf16, name="hTb")
                nc.vector.tensor_copy(hTb[:], hT[:])

                # gather h_src rows for all slots
                hs_all = hspool.tile([P, T * H], bf16, name="hs_all")
                for t in range(T):
                    nc.gpsimd.indirect_dma_start(
                        out=hs_all[:, t * H:(t + 1) * H], out_offset=None,
                        in_=h_rows[:],
                        in_offset=IndirectOffsetOnAxis(
                            ap=srcg_sb[:, t:t + 1], axis=0))

                # Per 256-node pair: C-build (one [32]x[32,512] matmul per
                # quadrant, two quadrants per 2-bank psum tile; psum cols are
                # (k, color) so C evicts k-major), then agg with 256
                # CONTIGUOUS moving columns per w2 weight-load.
                aggps = ps_agg.tile([H, NS], f32, name="aggps", tag="a")
                ei = 0
                for pr in range(NCHUNK // 2):
                    cc = cpool.tile([H, 256 * H], fp8, name="cc")
                    ccv = cc[:].rearrange("p (k n) -> p k n", k=H)
                    for tt in range(16):
                        t = pr * 16 + tt
                        for qp in range(2):
                            cps = ps_c.tile([H, 1024], f32, name="cps")
                            for qi in range(2):
                                q = qp * 2 + qi
                                nc.tensor.matmul(
                                    cps[:, qi * 512:(qi + 1) * 512],
                                    hs_all[32 * q:32 * (q + 1),
                                           t * H:(t + 1) * H],
                                    ev_all[32 * q:32 * (q + 1),
                                           t * 512:(t + 1) * 512],
                                    start=True, stop=True,
                                    tile_position=(32 * q, 0),
                                    skip_group_check=True)
                            cpsv = cps[:].rearrange(
                                "p (qi c k) -> p qi k c", qi=2, c=4)
                            for qi in range(2):
                                nb = tt * 16 + (qp * 2 + qi) * 4
                                src = cpsv[:, qi]
                                dst = ccv[:, :, nb:nb + 4]
                                if ei % 2 == 0:
                                    nc.vector.tensor_copy(dst, src)
                                else:
                                    nc.scalar.copy(dst, src)
                                ei += 1
                    if pr == 0:
                        # hbar[h, n] = scatter-mean of h_src (e2_b term);
                        # placed here so it doesn't stall on the gathers.
                        hbps = ps_hb.tile([H, NS], f32, name="hbps")
                        for t in range(T):
                            nc.tensor.matmul(
                                hbps[:, t * 16:(t + 1) * 16],
                                hs_all[:, t * H:(t + 1) * H],
                                smat_sb[:, t * 16:(t + 1) * 16],
                                start=True, stop=True,
                                skip_group_check=True)
                        hbarT = spool.tile([H, NS], bf16, name="hbarT")
                        nc.vector.tensor_copy(hbarT[:], hbps[:])
                    dst_ap = aggps[:, pr * 256:(pr + 1) * 256]
                    for k in range(H):
                        nc.tensor.matmul(dst_ap, w2_sb[:, k * P:(k + 1) * P],
                                         cc[:, k * 256:(k + 1) * 256],
                                         start=(k == 0),
                                         stop=False, skip_group_check=True)
                    nc.tensor.matmul(dst_ap, b2_sb[l][:],
                                     hbarT[:, pr * 256:(pr + 1) * 256],
                                     start=False, stop=False,
                                     skip_group_check=True)
                    nc.tensor.matmul(dst_ap, rw_sb[l][:],
                                     hTb[:, pr * 256:(pr + 1) * 256],
                                     start=False, stop=True,
                                     skip_group_check=True)

                # next layer's ev overlaps this layer's BN AllReduce wait
                if l < L - 1:
                    ev_next = gen_ev(l + 1)

                # BN stats: global sum & sum-of-squares over nodes
                stats = statp.tile([H, 2], f32, name="stats")
                nc.vector.tensor_reduce(stats[:, 0:1], aggps[:],
                                        axis=mybir.AxisListType.X, op=ALU.add)
                trash = spool.tile([H, NS], f32, name="trash")
                nc.scalar.activation(trash[:], aggps[:], AXF.Square,
                                     accum_out=stats[:, 1:2])
                st_in = dramp.tile([H, 2], f32, name="st_in")
                nc.sync.dma_start(st_in[:], stats[:])
                st_out = dramp.tile([H, 2], f32, name="st_out",
                                    addr_space="Shared")
                nc.gpsimd.collective_compute(
                    "AllReduce", ALU.add, replica_groups=groups,
                    ins=[st_in.opt()], outs=[st_out.opt()])
                stats2 = statp.tile([H, 2], f32, name="stats2")
                nc.sync.dma_start(stats2[:], st_out[:])

                mu = statp.tile([H, 1], f32, name="mu")
                nc.scalar.mul(mu[:], stats2[:, 0:1], 1.0 / N)
                ex2 = statp.tile([H, 1], f32, name="ex2")
                nc.scalar.mul(ex2[:], stats2[:, 1:2], 1.0 / N)
                musq = statp.tile([H, 1], f32, name="musq")
                nc.vector.tensor_mul(musq[:], mu[:], mu[:])
                var = statp.tile([H, 1], f32, name="var")
                nc.vector.tensor_tensor(out=var[:], in0=ex2[:], in1=musq[:],
                                        op=ALU.subtract)
                std = statp.tile([H, 1], f32, name="std")
                nc.scalar.activation(std[:], var[:], AXF.Sqrt,
                                     bias=eps_sb[:, 0:1])
                rstd = statp.tile([H, 1], f32, name="rstd")
                nc.vector.reciprocal(rstd[:], std[:])
                scal = statp.tile([H, 1], f32, name="scal")
                nc.vector.tensor_mul(scal[:], rstd[:], bng_sb[l][:])
                mscal = statp.tile([H, 1], f32, name="mscal")
                nc.vector.tensor_mul(mscal[:], mu[:], scal[:])
                shift = statp.tile([H, 1], f32, name="shift")
                nc.vector.tensor_tensor(out=shift[:], in0=bnb_sb[l][:],
                                        in1=mscal[:], op=ALU.subtract)

                relu_o = spool.tile([H, NS], f32, name="relu_o")
                nc.scalar.activation(relu_o[:], aggps[:], AXF.Relu,
                                     bias=shift[:, 0:1], scale=scal[:, 0:1])
                nc.vector.tensor_add(hT[:], hT[:], relu_o[:])

                # write updated slice (rows, bf16); AllGather except last layer
                if l < L - 1:
                    hsl = dramp.tile([NS, H], bf16, name="hsl")
                    for j in range(NS // P):
                        pst = ps_ev.tile([P, P], f32, name="hup_ps", tag="e")
                        nc.tensor.transpose(pst[:], hT[:, j * P:(j + 1) * P],
                                            idf_sb[:])
                        hrow = spool.tile([P, H], bf16, name="hup_row")
                        nc.scalar.copy(hrow[:], pst[:])
                        nc.sync.dma_start(hsl[j * P:(j + 1) * P, :], hrow[:])
                    hfull = dramp.tile([N, H], bf16, name="hfull")
                    nc.gpsimd.collective_compute(
                        "AllGather", ALU.bypass, replica_groups=groups,
                        ins=[hsl.opt()], outs=[hfull.opt()])
                    h_rows = hfull
                else:
                    for j in range(NS // P):
                        pst = ps_ev.tile([P, P], f32, name="hup_ps", tag="e")
                        nc.tensor.transpose(pst[:], hT[:, j * P:(j + 1) * P],
                                            idf_sb[:])
                        nc.scalar.copy(hrow_last[j][:], pst[:])

            # ---- head: sharded global-mean-pool + AllReduce + MLP ----
            with tc.tile_pool(name="headp", bufs=1) as headp:
                pmat_sb = headp.tile([P, 4 * G], bf16, bufs=1)
                nc.sync.dma_start(pmat_sb[:], pmat_d[:])
                hw1_sb = headp.tile([H, H], bf16, bufs=1)
                nc.sync.dma_start(hw1_sb[:], hw1_d[:])
                hb1_sb = headp.tile([H, 1], f32, bufs=1)
                nc.sync.dma_start(hb1_sb[:], hb1_d[:])
                hw2_sb = headp.tile([H, 1], bf16, bufs=1)
                nc.sync.dma_start(hw2_sb[:], hw2_d[:])
                hb2_sb = headp.tile([1, 1], f32, bufs=1)
                nc.sync.dma_start(hb2_sb[:], hb2_d[:])

                ps_pool = ps_agg.tile([H, G], f32, name="pool_ps", tag="a")
                for j in range(NS // P):
                    nc.tensor.matmul(ps_pool[:], hrow_last[j][:],
                                     pmat_sb[:, j * G:(j + 1) * G],
                                     start=(j == 0), stop=(j == NS // P - 1))
                pool_part = headp.tile([H, G], f32, name="pool_part")
                nc.vector.tensor_copy(pool_part[:], ps_pool[:])
                pl_in = dramp.tile([H, G], f32, name="pl_in")
                nc.sync.dma_start(pl_in[:], pool_part[:])
                pl_out = dramp.tile([H, G], f32, name="pl_out",
                                    addr_space="Shared")
                nc.gpsimd.collective_compute(
                    "AllReduce", ALU.add, replica_groups=groups,
                    ins=[pl_in.opt()], outs=[pl_out.opt()])
                pool_f = headp.tile([H, G], f32, name="pool_f")
                nc.sync.dma_start(pool_f[:], pl_out[:])
                pooledT = headp.tile([H, G], bf16, name="pooledT")
                nc.vector.tensor_copy(pooledT[:], pool_f[:])

                ps_z = ps_ev.tile([H, G], f32, name="z_ps", tag="e")
                nc.tensor.matmul(ps_z[:], hw1_sb[:], pooledT[:],
                                 start=True, stop=True)
                z = headp.tile([H, G], bf16, name="z")
                nc.scalar.activation(z[:], ps_z[:], AXF.Relu,
                                     bias=hb1_sb[:, 0:1])
                ps_y = ps_ev.tile([1, G], f32, name="y_ps", tag="e")
                nc.tensor.matmul(ps_y[:], hw2_sb[:], z[:], start=True,
                                 stop=True)
                ysb = headp.tile([1, G], f32, name="ysb")
                nc.vector.tensor_scalar_add(ysb[:], ps_y[:],
                                            hb2_sb[0:1, 0:1])
                nc.sync.dma_start(y_d[:], ysb[:])

    nc.compile()
    return nc


# ----------------------------------------------------------------------------
# Entry point
# ----------------------------------------------------------------------------

def kernel(**inputs):
    inp = {k: np.asarray(v) for k, v in inputs.items()}
    cores = _preprocess(inp["edge_index"], inp["edge_attr"])

    bf = lambda a: np.ascontiguousarray(np.asarray(a, np.float32)).astype(BF16)
    f32 = lambda a: np.ascontiguousarray(np.asarray(a, np.float32))

    e1w = np.concatenate(
        [np.asarray(inp["e1_w"], np.float32),
         np.asarray(inp["e1_b"], np.float32)[:, None, :]], axis=1)  # [L,17,128]
    # e2w_stack[l][h, k*128+o] = e2_w[l][k, h*128+o]
    w2s = np.asarray(inp["e2_w"], np.float32).reshape(L, H, H, H) \
        .transpose(0, 2, 1, 3).reshape(L, H, H * H)
    b2s = np.asarray(inp["e2_b"], np.float32).reshape(L, H, H)  # [h, o]
    xa = np.concatenate([np.asarray(inp["x"], np.float32).T,
                         np.ones((1, N), np.float32)], 0)  # [65, N]
    nw = np.concatenate([np.asarray(inp["node_w"], np.float32),
                         np.asarray(inp["node_b"], np.float32)[None, :]], 0)

    batch = np.asarray(inp["batch"], np.int64)
    cnt = np.bincount(batch, minlength=G).astype(np.float32)
    Pm = np.zeros((N, G), np.float32)
    Pm[np.arange(N), batch] = 1.0 / np.maximum(cnt, 1.0)[batch]

    shared = dict(
        e1w=bf(e1w), w2s=bf(w2s), b2s=bf(b2s), rw=bf(inp["root_w"]),
        bng=f32(inp["bn_g"])[:, :, None], bnb=f32(inp["bn_b"])[:, :, None],
        nw=bf(nw), hw1=bf(inp["head_w1"]), hb1=f32(inp["head_b1"])[:, None],
        hw2=bf(inp["head_w2"]), hb2=f32(inp["head_b2"])[None, :],
        idf=np.eye(P, dtype=np.float32),
    )

    in_maps = []
    for c in range(NC):
        cd = cores[c]
        m = dict(shared)
        m["ea"] = bf(cd["eaT"])
        m["srcg"] = np.ascontiguousarray(cd["srcg"])
        m["mscale"] = f32(cd["mscale"])
        m["smat"] = bf(cd["smat"])
        m["xs"] = bf(xa[:, c * NS:(c + 1) * NS])
        pm = np.zeros((P, 4 * G), np.float32)
        for j in range(NS // P):
            pm[:, j * G:(j + 1) * G] = Pm[c * NS + j * P: c * NS + (j + 1) * P]
        m["pmat"] = bf(pm)
        in_maps.append(m)

    nc = _build()
    import os
    trace = os.environ.get("KERNEL_TRACE", "0") == "1"
    res = run_bass_kernel_spmd(nc, in_maps, list(range(NC)), trace=trace)
    if trace and res.exec_time_ns is not None:
        print(f"HW exec time: {res.exec_time_ns} ns")
    y = np.asarray(res.results[0]["y"], np.float32).reshape(G)
    return y


# revision 22
# speedup vs baseline: 1.4445x; 1.0981x over previous
"""DMPNN (NNConv edge-network message passing) Trainium2 kernel, 8-core SPMD.

Algorithm: instead of materializing per-edge [H,H] weights (the reference's
W_e = relu(ea@e1)@e2, then msg_e = h_src W_e, scatter-mean), contract edges
into dst nodes FIRST via per-node outer products:

  C[n, (k,h)] = sum_{e->n} ev[e,k] * h_src[e,h] / deg_n      (tiny PE matmuls)
  aggT[o, n]  = sum_{k,h} e2w[k,h,o] * C[n,(k,h)]            (dense PE matmul)
              + hbar_n @ e2b + h @ root_w                     (bias + root)

This cuts the big contraction from E to N columns and avoids any E x 16384
intermediate. Per-node matmuls use 32-row PE array tiling: each dst node
group of 4 ("quadrant", 32 slots) does ONE [32]x[32,512] matmul with
color-masked ev (mask zeroes other nodes' rows; applied free during the
relu eviction).

Sharding: dst-node range per core (512 nodes). Per layer: AllReduce of BN
stats + AllGather of updated node features.
"""

import numpy as np
import ml_dtypes

import concourse.bass as bass
import concourse.tile as tile
import concourse.mybir as mybir
from concourse import bacc
from concourse.bass import IndirectOffsetOnAxis
from concourse.bass_utils import run_bass_kernel_spmd

BF16 = ml_dtypes.bfloat16

N, E, F_NODE, F_EDGE, H, L, G = 4096, 12288, 64, 16, 128, 4, 256
NC = 8
NS = N // NC            # nodes per core (512)
P = 128
T = 32                  # slot tiles per core (32 x 128 slots)
NQ = 4 * T              # quadrants (one per 4 nodes)
NCHUNK = 4              # agg chunks of 128 nodes
BN_EPS = 1e-5
AXF = mybir.ActivationFunctionType
ALU = mybir.AluOpType


# ----------------------------------------------------------------------------
# Host preprocessing
# ----------------------------------------------------------------------------

def _preprocess(edge_index, edge_attr):
    src = np.asarray(edge_index[0], dtype=np.int64)
    dst = np.asarray(edge_index[1], dtype=np.int64)
    ea = np.asarray(edge_attr, dtype=np.float32)
    deg = np.bincount(dst, minlength=N).astype(np.float32)
    inv_deg = np.where(deg > 0, 1.0 / np.maximum(deg, 1.0), 0.0).astype(np.float32)

    # edges grouped by dst
    order = np.argsort(dst, kind="stable")
    starts = np.searchsorted(dst[order], np.arange(N))
    ends = np.searchsorted(dst[order], np.arange(N), side="right")

    cores = []
    for c in range(NC):
        eaT = np.zeros((17, T * P), np.float32)
        srcg = np.zeros((P, T), np.int32)
        mscale = np.zeros((P, T * 4), np.float32)
        smat = np.zeros((P, T * 16), np.float32)
        for t in range(T):
            for q in range(4):
                p0 = 32 * q
                fill = 0
                for cidx in range(4):
                    n_local = 16 * t + 4 * q + cidx
                    n_glob = c * NS + n_local
                    es = order[starts[n_glob]:ends[n_glob]]
                    k = len(es)
                    assert fill + k <= 32, (c, t, q, fill, k)
                    sl = slice(p0 + fill, p0 + fill + k)
                    eaT[:F_EDGE, t * P + p0 + fill: t * P + p0 + fill + k] = ea[es].T
                    eaT[F_EDGE, t * P + p0 + fill: t * P + p0 + fill + k] = 1.0
                    srcg[sl, t] = src[es]
                    mscale[sl, t * 4 + cidx] = inv_deg[n_glob]
                    smat[sl, t * 16 + 4 * q + cidx] = inv_deg[n_glob]
                    fill += k
        cores.append(dict(eaT=eaT, srcg=srcg, mscale=mscale, smat=smat))
    return cores


# ----------------------------------------------------------------------------
# Device program
# ----------------------------------------------------------------------------

def _build(has_b2):
    f32 = mybir.dt.float32
    bf16 = mybir.dt.bfloat16
    fp8 = mybir.dt.float8e4
    i32 = mybir.dt.int32
    nc = bacc.Bacc("TRN2", target_bir_lowering=False, debug=False, num_devices=NC)

    def din(name, shape, dt=bf16):
        return nc.dram_tensor(name, shape, dt, kind="ExternalInput")

    ea_d = din("ea", [17, T * P])
    srcg_d = din("srcg", [P, T], i32)
    mscale_d = din("mscale", [P, T * 4], f32)
    smat_d = din("smat", [P, T * 16])
    e1w_d = din("e1w", [L, 17, H])
    w2s_d = din("w2s", [L, H, H * H])      # e2w_stack[l][h, k*128+o]
    b2s_d = din("b2s", [L, H, H])          # e2_b as [h, o]
    rw_d = din("rw", [L, H, H])            # root_w as [h, o]
    bng_d = din("bng", [L, H, 1], f32)
    bnb_d = din("bnb", [L, H, 1], f32)
    xs_d = din("xs", [65, NS])
    nw_d = din("nw", [65, H])
    pmat_d = din("pmat", [P, 4 * G])
    hw1_d = din("hw1", [H, H])
    hb1_d = din("hb1", [H, 1], f32)
    hw2_d = din("hw2", [H, 1])
    hb2_d = din("hb2", [1, 1], f32)
    idf_d = din("idf", [P, P], f32)
    y_d = nc.dram_tensor("y", [1, G], f32, kind="ExternalOutput")

    groups = [list(range(NC))]

    with tile.TileContext(nc) as tc:
        with tc.tile_pool(name="const", bufs=1) as const, \
             tc.tile_pool(name="persist", bufs=1) as persist, \
             tc.tile_pool(name="w2pool", bufs=1) as w2pool, \
             tc.tile_pool(name="cpool", bufs=2) as cpool, \
             tc.tile_pool(name="evpool", bufs=2) as evpool, \
             tc.tile_pool(name="hspool", bufs=1) as hspool, \
             tc.tile_pool(name="spool", bufs=2) as spool, \
             tc.tile_pool(name="stat", bufs=2) as statp, \
             tc.tile_pool(name="psc", bufs=2, space="PSUM") as ps_c, \
             tc.tile_pool(name="psagg", bufs=1, space="PSUM") as ps_agg, \
             tc.tile_pool(name="pshb", bufs=1, space="PSUM") as ps_hb, \
             tc.tile_pool(name="psev", bufs=2, space="PSUM") as ps_ev, \
             tc.tile_pool(name="dramp", bufs=2, space="DRAM") as dramp:

            # ---- persistent constants (encoder-critical ones first so the
            # first collective fires early and barrier skew overlaps the
            # remaining loads) ----
            xs_sb = const.tile([65, NS], bf16)
            nc.sync.dma_start(xs_sb[:], xs_d[:])
            nw_sb = const.tile([65, H], bf16)
            nc.sync.dma_start(nw_sb[:], nw_d[:])
            idf_sb = const.tile([P, P], f32)
            nc.sync.dma_start(idf_sb[:], idf_d[:])
            ea_sb = const.tile([17, T * P], bf16)
            srcg_sb = const.tile([P, T], i32)
            mscale_sb = const.tile([P, T * 4], f32)
            smat_sb = const.tile([P, T * 16], bf16)
            e1w_sb, b2_sb, rw_sb, bng_sb, bnb_sb = [], [], [], [], []
            for l in range(L):
                t_ = const.tile([17, H], bf16, name=f"e1w_{l}")
                nc.sync.dma_start(t_[:], e1w_d[l])
                e1w_sb.append(t_)
                t_ = const.tile([H, H], bf16, name=f"b2_{l}")
                nc.sync.dma_start(t_[:], b2s_d[l])
                b2_sb.append(t_)
                t_ = const.tile([H, H], bf16, name=f"rw_{l}")
                nc.sync.dma_start(t_[:], rw_d[l])
                rw_sb.append(t_)
                t_ = const.tile([H, 1], f32, name=f"bng_{l}")
                nc.sync.dma_start(t_[:], bng_d[l])
                bng_sb.append(t_)
                t_ = const.tile([H, 1], f32, name=f"bnb_{l}")
                nc.sync.dma_start(t_[:], bnb_d[l])
                bnb_sb.append(t_)
            eps_sb = const.tile([H, 1], f32)
            nc.vector.memset(eps_sb[:], BN_EPS)

            hT = persist.tile([H, NS], f32)      # own node features, [h, n]
            hrow_last = [persist.tile([P, H], bf16, name=f"hrl_{j}")
                         for j in range(4)]

            # ---- node encoder (own slice only) + AllGather ----
            hsl0 = dramp.tile([NS, H], bf16, name="hsl0", bufs=1)
            with tc.tile_pool(name="encp", bufs=2) as encp:
                for j in range(NS // P):
                    ps = ps_ev.tile([P, H], f32, name="enc_ps", tag="e")
                    nc.tensor.matmul(ps[:], xs_sb[:, j * P:(j + 1) * P], nw_sb[:],
                                     start=True, stop=True)
                    tmp = encp.tile([P, H], f32, name="enc_tmp")
                    nc.vector.tensor_copy(tmp[:], ps[:])
                    hrow = encp.tile([P, H], bf16, name="enc_row")
                    nc.scalar.copy(hrow[:], ps[:])
                    nc.sync.dma_start(hsl0[j * P:(j + 1) * P, :], hrow[:])
                    ps2 = ps_ev.tile([P, P], f32, name="enc_ps2", tag="e")
                    nc.tensor.transpose(ps2[:], tmp[:], idf_sb[:])
                    nc.scalar.copy(hT[:, j * P:(j + 1) * P], ps2[:])
            hfull0 = dramp.tile([N, H], bf16, name="hfull0", bufs=1)
            nc.gpsimd.collective_compute(
                "AllGather", ALU.bypass, replica_groups=groups,
                ins=[hsl0.opt()], outs=[hfull0.opt()])
            h_rows = hfull0
            nc.sync.dma_start(ea_sb[:], ea_d[:])
            nc.sync.dma_start(srcg_sb[:], srcg_d[:])
            nc.sync.dma_start(mscale_sb[:], mscale_d[:])
            nc.sync.dma_start(smat_sb[:], smat_d[:])

            # ev generation + color-masked eviction (scaled by 1/deg).
            # Depends only on edge_attr, so layer l+1's ev runs during
            # layer l's BN AllReduce wait.
            def gen_ev(l):
                # ev_all tile-block columns are (k, color)-interleaved so the
                # C-build psum comes out k-major.
                ev_all = evpool.tile([P, T * 512], bf16, name="ev_all")
                for t in range(T):
                    evps = ps_ev.tile([P, H], f32, name="evps", tag="e")
                    nc.tensor.matmul(evps[:], ea_sb[:17, t * P:(t + 1) * P],
                                     e1w_sb[l][:], start=True, stop=True)
                    for cidx in range(4):
                        dst_ap = ev_all[:, t * 512 + cidx * P:
                                        t * 512 + (cidx + 1) * P]
                        sc = mscale_sb[:, t * 4 + cidx: t * 4 + cidx + 1]
                        if cidx % 2 == 0:
                            nc.vector.tensor_scalar(
                                out=dst_ap, in0=evps[:], scalar1=sc,
                                scalar2=0.0, op0=ALU.mult, op1=ALU.max)
                        else:
                            nc.scalar.activation(dst_ap, evps[:], AXF.Relu,
                                                 scale=sc)
                return ev_all

            ev_next = gen_ev(0)

            # ---- layers ----
            for l in range(L):
                ev_all = ev_next
                w2_sb = w2pool.tile([H, H * H], bf16, name="w2")
                nc.sync.dma_start(w2_sb[:], w2s_d[l])

                hTb = spool.tile([H, NS], bf16, name="hTb")
                nc.vector.tensor_copy(hTb[:], hT[:])

                # gather h_src rows for all slots
                hs_all = hspool.tile([P, T * H], bf16, name="hs_all")
                for t in range(T):
                    nc.gpsimd.indirect_dma_start(
                        out=hs_all[:, t * H:(t + 1) * H], out_offset=None,
                        in_=h_rows[:],
                        in_offset=IndirectOffsetOnAxis(
                            ap=srcg_sb[:, t:t + 1], axis=0))

                # Per 256-node pair: C-build (one [32]x[32,512] matmul per
                # quadrant, two quadrants per 2-bank psum tile; psum cols are
                # (k, color) so C evicts k-major), then agg with 256
                # CONTIGUOUS moving columns per w2 weight-load.
                aggps = ps_agg.tile([H, NS], f32, name="aggps", tag="a")
                ei = 0
                for pr in range(NCHUNK // 2):
                    cc = cpool.tile([H, 256 * H], fp8, name="cc")
                    ccv = cc[:].rearrange("p (k n) -> p k n", k=H)
                    for tt in range(16):
                        t = pr * 16 + tt
                        for qp in range(2):
                            cps = ps_c.tile([H, 1024], f32, name="cps")
                            for qi in range(2):
                                q = qp * 2 + qi
                                nc.tensor.matmul(
                                    cps[:, qi * 512:(qi + 1) * 512],
                                    hs_all[32 * q:32 * (q + 1),
                                           t * H:(t + 1) * H],
                                    ev_all[32 * q:32 * (q + 1),
                                           t * 512:(t + 1) * 512],
                                    start=True, stop=True,
                                    tile_position=(32 * q, 0),
                                    skip_group_check=True)
                            cpsv = cps[:].rearrange(
                                "p (qi c k) -> p qi k c", qi=2, c=4)
                            for qi in range(2):
                                nb = tt * 16 + (qp * 2 + qi) * 4
                                src = cpsv[:, qi]
                                dst = ccv[:, :, nb:nb + 4]
                                if ei % 2 == 0:
                                    nc.vector.tensor_copy(dst, src)
                                else:
                                    nc.scalar.copy(dst, src)
                                ei += 1
                    if pr == 0 and has_b2:
                        # hbar[h, n] = scatter-mean of h_src (e2_b term);
                        # placed here so it doesn't stall on the gathers.
                        hbps = ps_hb.tile([H, NS], f32, name="hbps")
                        for t in range(T):
                            nc.tensor.matmul(
                                hbps[:, t * 16:(t + 1) * 16],
                                hs_all[:, t * H:(t + 1) * H],
                                smat_sb[:, t * 16:(t + 1) * 16],
                                start=True, stop=True,
                                skip_group_check=True)
                        hbarT = spool.tile([H, NS], bf16, name="hbarT")
                        nc.vector.tensor_copy(hbarT[:], hbps[:])
                    dst_ap = aggps[:, pr * 256:(pr + 1) * 256]
                    for k in range(H):
                        nc.tensor.matmul(dst_ap, w2_sb[:, k * P:(k + 1) * P],
                                         cc[:, k * 256:(k + 1) * 256],
                                         start=(k == 0),
                                         stop=False, skip_group_check=True)
                    if has_b2:
                        nc.tensor.matmul(dst_ap, b2_sb[l][:],
                                         hbarT[:, pr * 256:(pr + 1) * 256],
                                         start=False, stop=False,
                                         skip_group_check=True)
                    nc.tensor.matmul(dst_ap, rw_sb[l][:],
                                     hTb[:, pr * 256:(pr + 1) * 256],
                                     start=False, stop=True,
                                     skip_group_check=True)

                # next layer's ev overlaps this layer's BN AllReduce wait
                if l < L - 1:
                    ev_next = gen_ev(l + 1)

                # BN stats: global sum & sum-of-squares over nodes
                stats = statp.tile([H, 2], f32, name="stats")
                nc.vector.tensor_reduce(stats[:, 0:1], aggps[:],
                                        axis=mybir.AxisListType.X, op=ALU.add)
                trash = spool.tile([H, NS], f32, name="trash")
                nc.scalar.activation(trash[:], aggps[:], AXF.Square,
                                     accum_out=stats[:, 1:2])
                st_in = dramp.tile([H, 2], f32, name="st_in")
                nc.sync.dma_start(st_in[:], stats[:])
                st_out = dramp.tile([H, 2], f32, name="st_out",
                                    addr_space="Shared")
                nc.gpsimd.collective_compute(
                    "AllReduce", ALU.add, replica_groups=groups,
                    ins=[st_in.opt()], outs=[st_out.opt()])
                stats2 = statp.tile([H, 2], f32, name="stats2")
                nc.sync.dma_start(stats2[:], st_out[:])

                mu = statp.tile([H, 1], f32, name="mu")
                nc.scalar.mul(mu[:], stats2[:, 0:1], 1.0 / N)
                ex2 = statp.tile([H, 1], f32, name="ex2")
                nc.scalar.mul(ex2[:], stats2[:, 1:2], 1.0 / N)
                musq = statp.tile([H, 1], f32, name="musq")
                nc.vector.tensor_mul(musq[:], mu[:], mu[:])
                var = statp.tile([H, 1], f32, name="var")
                nc.vector.tensor_tensor(out=var[:], in0=ex2[:], in1=musq[:],
                                        op=ALU.subtract)
                std = statp.tile([H, 1], f32, name="std")
                nc.scalar.activation(std[:], var[:], AXF.Sqrt,
                                     bias=eps_sb[:, 0:1])
                rstd = statp.tile([H, 1], f32, name="rstd")
                nc.vector.reciprocal(rstd[:], std[:])
                scal = statp.tile([H, 1], f32, name="scal")
                nc.vector.tensor_mul(scal[:], rstd[:], bng_sb[l][:])
                mscal = statp.tile([H, 1], f32, name="mscal")
                nc.vector.tensor_mul(mscal[:], mu[:], scal[:])
                shift = statp.tile([H, 1], f32, name="shift")
                nc.vector.tensor_tensor(out=shift[:], in0=bnb_sb[l][:],
                                        in1=mscal[:], op=ALU.subtract)

                relu_o = spool.tile([H, NS], f32, name="relu_o")
                nc.scalar.activation(relu_o[:], aggps[:], AXF.Relu,
                                     bias=shift[:, 0:1], scale=scal[:, 0:1])
                nc.vector.tensor_add(hT[:], hT[:], relu_o[:])

                # write updated slice (rows, bf16); AllGather except last layer
                if l < L - 1:
                    hsl = dramp.tile([NS, H], bf16, name="hsl")
                    for j in range(NS // P):
                        pst = ps_ev.tile([P, P], f32, name="hup_ps", tag="e")
                        nc.tensor.transpose(pst[:], hT[:, j * P:(j + 1) * P],
                                            idf_sb[:])
                        hrow = spool.tile([P, H], bf16, name="hup_row")
                        nc.scalar.copy(hrow[:], pst[:])
                        nc.sync.dma_start(hsl[j * P:(j + 1) * P, :], hrow[:])
                    hfull = dramp.tile([N, H], bf16, name="hfull")
                    nc.gpsimd.collective_compute(
                        "AllGather", ALU.bypass, replica_groups=groups,
                        ins=[hsl.opt()], outs=[hfull.opt()])
                    h_rows = hfull
                else:
                    for j in range(NS // P):
                        pst = ps_ev.tile([P, P], f32, name="hup_ps", tag="e")
                        nc.tensor.transpose(pst[:], hT[:, j * P:(j + 1) * P],
                                            idf_sb[:])
                        nc.scalar.copy(hrow_last[j][:], pst[:])

            # ---- head: sharded global-mean-pool + AllReduce + MLP ----
            with tc.tile_pool(name="headp", bufs=1) as headp:
                pmat_sb = headp.tile([P, 4 * G], bf16, bufs=1)
                nc.sync.dma_start(pmat_sb[:], pmat_d[:])
                hw1_sb = headp.tile([H, H], bf16, bufs=1)
                nc.sync.dma_start(hw1_sb[:], hw1_d[:])
                hb1_sb = headp.tile([H, 1], f32, bufs=1)
                nc.sync.dma_start(hb1_sb[:], hb1_d[:])
                hw2_sb = headp.tile([H, 1], bf16, bufs=1)
                nc.sync.dma_start(hw2_sb[:], hw2_d[:])
                hb2_sb = headp.tile([1, 1], f32, bufs=1)
                nc.sync.dma_start(hb2_sb[:], hb2_d[:])

                ps_pool = ps_agg.tile([H, G], f32, name="pool_ps", tag="a")
                for j in range(NS // P):
                    nc.tensor.matmul(ps_pool[:], hrow_last[j][:],
                                     pmat_sb[:, j * G:(j + 1) * G],
                                     start=(j == 0), stop=(j == NS // P - 1))
                pool_part = headp.tile([H, G], f32, name="pool_part")
                nc.vector.tensor_copy(pool_part[:], ps_pool[:])
                pl_in = dramp.tile([H, G], f32, name="pl_in")
                nc.sync.dma_start(pl_in[:], pool_part[:])
                pl_out = dramp.tile([H, G], f32, name="pl_out",
                                    addr_space="Shared")
                nc.gpsimd.collective_compute(
                    "AllReduce", ALU.add, replica_groups=groups,
                    ins=[pl_in.opt()], outs=[pl_out.opt()])
                pool_f = headp.tile([H, G], f32, name="pool_f")
                nc.sync.dma_start(pool_f[:], pl_out[:])
                pooledT = headp.tile([H, G], bf16, name="pooledT")
                nc.vector.tensor_copy(pooledT[:], pool_f[:])

                ps_z = ps_ev.tile([H, G], f32, name="z_ps", tag="e")
                nc.tensor.matmul(ps_z[:], hw1_sb[:], pooledT[:],
                                 start=True, stop=True)
                z = headp.tile([H, G], bf16, name="z")
                nc.scalar.activation(z[:], ps_z[:], AXF.Relu,
                                     bias=hb1_sb[:, 0:1])
                ps_y = ps_ev.tile([1, G], f32, name="y_ps", tag="e")
                nc.tensor.matmul(ps_y[:], hw2_sb[:], z[:], start=True,
                                 stop=True)
                ysb = headp.tile([1, G], f32, name="ysb")
                nc.vector.tensor_scalar_add(ysb[:], ps_y[:],
                                            hb2_sb[0:1, 0:1])
                nc.sync.dma_start(y_d[:], ysb[:])

    nc.compile()
    return nc


# ----------------------------------------------------------------------------
# Entry point
# ----------------------------------------------------------------------------

def kernel(**inputs):
    inp = {k: np.asarray(v) for k, v in inputs.items()}
    cores = _preprocess(inp["edge_index"], inp["edge_attr"])

    bf = lambda a: np.ascontiguousarray(np.asarray(a, np.float32)).astype(BF16)
    f32 = lambda a: np.ascontiguousarray(np.asarray(a, np.float32))

    e1w = np.concatenate(
        [np.asarray(inp["e1_w"], np.float32),
         np.asarray(inp["e1_b"], np.float32)[:, None, :]], axis=1)  # [L,17,128]
    # e2w_stack[l][h, k*128+o] = e2_w[l][k, h*128+o]
    w2s = np.asarray(inp["e2_w"], np.float32).reshape(L, H, H, H) \
        .transpose(0, 2, 1, 3).reshape(L, H, H * H)
    b2s = np.asarray(inp["e2_b"], np.float32).reshape(L, H, H)  # [h, o]
    xa = np.concatenate([np.asarray(inp["x"], np.float32).T,
                         np.ones((1, N), np.float32)], 0)  # [65, N]
    nw = np.concatenate([np.asarray(inp["node_w"], np.float32),
                         np.asarray(inp["node_b"], np.float32)[None, :]], 0)

    batch = np.asarray(inp["batch"], np.int64)
    cnt = np.bincount(batch, minlength=G).astype(np.float32)
    Pm = np.zeros((N, G), np.float32)
    Pm[np.arange(N), batch] = 1.0 / np.maximum(cnt, 1.0)[batch]

    shared = dict(
        e1w=bf(e1w), w2s=bf(w2s), b2s=bf(b2s), rw=bf(inp["root_w"]),
        bng=f32(inp["bn_g"])[:, :, None], bnb=f32(inp["bn_b"])[:, :, None],
        nw=bf(nw), hw1=bf(inp["head_w1"]), hb1=f32(inp["head_b1"])[:, None],
        hw2=bf(inp["head_w2"]), hb2=f32(inp["head_b2"])[None, :],
        idf=np.eye(P, dtype=np.float32),
    )

    in_maps = []
    for c in range(NC):
        cd = cores[c]
        m = dict(shared)
        m["ea"] = bf(cd["eaT"])
        m["srcg"] = np.ascontiguousarray(cd["srcg"])
        m["mscale"] = f32(cd["mscale"])
        m["smat"] = bf(cd["smat"])
        m["xs"] = bf(xa[:, c * NS:(c + 1) * NS])
        pm = np.zeros((P, 4 * G), np.float32)
        for j in range(NS // P):
            pm[:, j * G:(j + 1) * G] = Pm[c * NS + j * P: c * NS + (j + 1) * P]
        m["pmat"] = bf(pm)
        in_maps.append(m)

    has_b2 = bool(np.any(np.asarray(inp["e2_b"], np.float32)))
    nc = _build(has_b2)
    import os
    trace = os.environ.get("KERNEL_TRACE", "0") == "1"
    res = run_bass_kernel_spmd(nc, in_maps, list(range(NC)), trace=trace)
    if trace and res.exec_time_ns is not None:
        print(f"HW exec time: {res.exec_time_ns} ns")
    y = np.asarray(res.results[0]["y"], np.float32).reshape(G)
    return y
